# revision 35
# baseline (speedup 1.0000x reference)
"""Trainium2 Bass kernel for segment-softmax attention (segment_reduce).

Computes, for row-sorted segment ids `index` (N rows, B segments):
    src  = tanh([x, ref] @ W + b)            # [N, 1]
    w    = segment_softmax(src, index)       # [N, 1]
    out  = segment_sum(w * x, index)         # [B, D]

Strategy (8 NeuronCores, SPMD, no collectives):
  - B segments are split into groups of 128; each core owns B/128/8
    contiguous groups, so shard boundaries align to segment boundaries
    and no cross-core reduction is needed.  Group row-ranges come from
    the host (sorted index), padded to a common chunk count C.
  - src = tanh(.) is in (-1,1), so exp never overflows and the segment
    max subtraction is dropped (identical up to float rounding).
  - Per 128-row chunk k of a group (on device):
      PE:  src column = Xt_k.T @ W1 + Rt_k.T @ W2        (psum [128,1])
      ACT: e = exp(tanh(src)) batched per group
      DVE: A[n,s] = e[n] * (idx[n] == s)  (one fused tensor_scalar)
      PE:  psum[128 segs, 129] += A.T @ [X_k | 1]        (col 128 = Z)
    evacuation: out = psum[:, :128] / (Z + 1e-16)  (DVE recip + ACT scale)
  - Value matmuls of group i run interleaved with the matvec matmuls of
    group i+2 (2-ahead software pipeline); psum accumulation alternates
    between two banks to keep consecutive matmuls pipelined.
  - Inputs are pre-quantized to bf16 on the host in the two layouts the
    PE needs (chunk-transposed for the matvec, row-major+ones column for
    the value pass); halves DMA traffic, rel-err ~3e-3 vs f32 reference.
"""

import numpy as np

N_CORES = 8
D = 128
SEG_PER_GROUP = 128  # psum partition dim = segments per group

_BF16_ONE = np.uint16(0x3F80)


def _f32_to_bf16_u16(a: np.ndarray) -> np.ndarray:
    """Round-to-nearest f32 -> bf16 bit pattern (uint16)."""
    a = np.ascontiguousarray(a, dtype=np.float32)
    u = a.view(np.uint32)
    rnd = ((u >> 16) & 1) + np.uint32(0x7FFF)
    return ((u + rnd) >> 16).astype(np.uint16)


def _build_graph(gpc: int, c_chunks: int):
    """Build the SPMD single-core graph (identical on all 8 cores)."""
    import concourse.bacc as bacc
    import concourse.mybir as mybir
    from concourse import tile
    from concourse.tile import add_dep_helper
    from contextlib import ExitStack

    dt = mybir.dt
    AF = mybir.ActivationFunctionType
    ALU = mybir.AluOpType

    C = c_chunks
    GC = gpc * C  # total chunks per core

    nc = bacc.Bacc(
        "TRN2",
        target_bir_lowering=False,
        debug=False,
        num_devices=N_CORES,
    )

    xtr = nc.dram_tensor("xtr", [128, GC * 128], dt.bfloat16, kind="ExternalInput").ap()
    rtr = nc.dram_tensor("rtr", [128, GC * 128], dt.bfloat16, kind="ExternalInput").ap()
    xrm = nc.dram_tensor("xrm", [128, GC * 129], dt.bfloat16, kind="ExternalInput").ap()
    idxg = nc.dram_tensor("idxg", [128, GC], dt.float32, kind="ExternalInput").ap()
    wco = nc.dram_tensor("wco", [128, 2], dt.bfloat16, kind="ExternalInput").ap()
    io2 = nc.dram_tensor("io2", [128, 128], dt.bfloat16, kind="ExternalInput").ap()
    out = nc.dram_tensor(
        "out", [gpc * SEG_PER_GROUP, 2 * (D + 1)], dt.float32, kind="ExternalOutput"
    ).ap()

    with tile.TileContext(nc) as tc, ExitStack() as ctx:
        cpool = ctx.enter_context(tc.tile_pool(name="consts", bufs=1))
        xtp = ctx.enter_context(tc.tile_pool(name="xtp", bufs=2))
        rtp = ctx.enter_context(tc.tile_pool(name="rtp", bufs=2))
        xmp = ctx.enter_context(tc.tile_pool(name="xmp", bufs=3))
        epool = ctx.enter_context(tc.tile_pool(name="e", bufs=3))
        apool = ctx.enter_context(tc.tile_pool(name="amat", bufs=24))
        opool = ctx.enter_context(tc.tile_pool(name="osb", bufs=4))
        zpool = ctx.enter_context(tc.tile_pool(name="zr", bufs=4))
        ps_s = ctx.enter_context(tc.tile_pool(name="pss", bufs=2, space="PSUM"))
        ps_o = ctx.enter_context(tc.tile_pool(name="pso", bufs=6, space="PSUM"))

        wt = cpool.tile([128, 2], dt.bfloat16)
        nc.sync.dma_start(wt[:], wco[:])
        it = cpool.tile([128, 128], dt.bfloat16)
        nc.sync.dma_start(it[:], io2[:])
        # whole per-core index array resident in SBUF (2KB/partition)
        ixall = cpool.tile([128, GC], dt.float32)
        nc.sync.dma_start(ixall[:], idxg[:])

        st = {}  # live tiles per pipeline stage

        def emit_load_and_src(g):
            xt = xtp.tile([128, C * 128], dt.bfloat16, tag="xt")
            nc.sync.dma_start(xt[:], xtr[:, g * C * 128:(g + 1) * C * 128])
            rt = rtp.tile([128, C * 128], dt.bfloat16, tag="rt")
            nc.sync.dma_start(rt[:], rtr[:, g * C * 128:(g + 1) * C * 128])
            xm = xmp.tile([128, C * 129], dt.bfloat16, tag="xm")
            nc.sync.dma_start(xm[:], xrm[:, g * C * 129:(g + 1) * C * 129])
            src = ps_s.tile([128, C], dt.float32, tag="src")
            st[g] = dict(xt=xt, rt=rt, xm=xm, src=src)

        def emit_src_chunk(g, k, after=None):
            s = st[g]
            mm = nc.tensor.matmul(
                s["src"][:, k:k + 1],
                s["xt"][:, k * 128:(k + 1) * 128],
                wt[:, 0:1],
                start=(k == 0),
                stop=False,
            )
            if after is not None:
                # ordering-only edge: spread the matvec matmuls between the
                # value matmuls instead of clustering at group boundaries
                add_dep_helper(mm.ins, after.ins, sync=False, reason="interleave")
            nc.tensor.matmul(
                s["src"][:, k:k + 1],
                s["rt"][:, k * 128:(k + 1) * 128],
                wt[:, 1:2],
                start=False,
                stop=(k == C - 1),
            )

        def emit_act(g):
            s = st[g]
            th = epool.tile([128, C], dt.float32, tag="th")
            nc.scalar.activation(th[:], s["src"][:], AF.Tanh)
            ee = epool.tile([128, C], dt.float32, tag="ee")
            nc.scalar.activation(ee[:], th[:], AF.Exp)
            s["ee"] = ee

        def emit_po_alloc(g):
            # two psum banks alternate per chunk so consecutive accumulating
            # matmuls never target the same bank (keeps fill/drain pipelined)
            st[g]["po"] = [
                ps_o.tile([128, 129], dt.float32, tag="po", name="po"),
                ps_o.tile([128, 129], dt.float32, tag="po", name="po"),
            ]

        def emit_val_chunk(g, k):
            s = st[g]
            amat = apool.tile([128, 128], dt.bfloat16, tag="amat")
            nc.vector.tensor_scalar(
                amat[:],
                it[:],
                ixall[:, g * C + k:g * C + k + 1],
                s["ee"][:, k:k + 1],
                op0=ALU.is_equal,
                op1=ALU.mult,
            )
            return nc.tensor.matmul(
                s["po"][k % 2][:],
                amat[:],
                s["xm"][:, k * 129:(k + 1) * 129],
                start=(k < 2),
                stop=(k >= C - 2),
            )

        def emit_evac(g):
            # evacuate both raw psum banks via the idle scalar engine; the
            # bank merge and Z-division happen in the host unshard, so the
            # DVE critical chain carries zero evacuation work
            s = st.pop(g)
            po_a, po_b = s["po"]
            ob = opool.tile([128, 2 * 129], dt.float32, tag="ob")
            nc.scalar.copy(ob[:, 0:129], po_a[:])
            nc.scalar.copy(ob[:, 129:258], po_b[:])
            nc.sync.dma_start(
                out[g * SEG_PER_GROUP:(g + 1) * SEG_PER_GROUP, :], ob[:]
            )

        # 2-ahead software pipeline: group i's value pass overlaps group
        # (i+2)'s load+matvec, so e(i+1) is always ready when the value
        # pass advances.
        for g in (0, 1):
            if g < gpc:
                emit_load_and_src(g)
                for k in range(C):
                    emit_src_chunk(g, k)
                emit_act(g)
        for i in range(gpc):
            emit_po_alloc(i)
            if i + 2 < gpc:
                emit_load_and_src(i + 2)
            last_vmm = None
            for k in range(C):
                if i + 2 < gpc:
                    emit_src_chunk(i + 2, k, after=last_vmm)
                last_vmm = emit_val_chunk(i, k)
            if i + 2 < gpc:
                emit_act(i + 2)
            emit_evac(i)

    nc.compile()
    return nc


_GRAPH_CACHE: dict = {}


def _get_graph(gpc: int, c_chunks: int):
    key = (gpc, c_chunks)
    if key not in _GRAPH_CACHE:
        _GRAPH_CACHE[key] = _build_graph(gpc, c_chunks)
    return _GRAPH_CACHE[key]


def _prepare_inputs(x, ref, index, batch_size, W, b):
    """Host-side sharding: group-aligned padding + bf16 layouts per core."""
    import concourse.mybir as mybir

    bf16 = mybir.dt.np(mybir.dt.bfloat16)

    x = np.ascontiguousarray(np.asarray(x, dtype=np.float32))
    ref = np.ascontiguousarray(np.asarray(ref, dtype=np.float32))
    idx = np.asarray(index).astype(np.int64).ravel()
    W = np.asarray(W, dtype=np.float32).reshape(-1)
    b_val = float(np.asarray(b, dtype=np.float32).reshape(-1)[0])

    n, d = x.shape
    assert d == D
    B = int(batch_size)
    ngroups = B // SEG_PER_GROUP
    assert B % SEG_PER_GROUP == 0 and ngroups % N_CORES == 0
    gpc = ngroups // N_CORES

    bounds = np.searchsorted(idx, np.arange(0, B + 1, SEG_PER_GROUP))
    rows_g = np.diff(bounds)
    C = max(1, int(np.ceil(rows_g.max() / 128)))
    R = C * 128

    offs = np.arange(R)[None, :]
    gidx = bounds[:-1, None] + offs  # [NG, R]
    valid = offs < rows_g[:, None]
    gidx_c = np.where(valid, np.minimum(gidx, n - 1), 0)

    # group-relative segment id; padding rows get 300 (never matches 0..127)
    idx_rel = np.where(
        valid,
        idx[gidx_c] - (np.arange(ngroups) * SEG_PER_GROUP)[:, None],
        300,
    ).astype(np.float32)

    xg = _f32_to_bf16_u16(x[gidx_c])  # [NG, R, D] u16
    rg = _f32_to_bf16_u16(ref[gidx_c])

    wco = np.zeros((128, 2), dtype=np.uint16)
    wco[:, 0] = _f32_to_bf16_u16(W[:128])
    wco[:, 1] = _f32_to_bf16_u16(W[128:256])
    wco = wco.view(bf16)

    io2 = np.broadcast_to(
        _f32_to_bf16_u16(np.arange(128, dtype=np.float32))[None, :], (128, 128)
    )
    io2 = np.ascontiguousarray(io2).view(bf16)

    in_maps = []
    for cid in range(N_CORES):
        sl = slice(cid * gpc, (cid + 1) * gpc)
        xc = xg[sl].reshape(gpc * C, 128, D)  # [chunks, row, d] u16
        rc = rg[sl].reshape(gpc * C, 128, D)

        xtr = np.ascontiguousarray(xc.transpose(2, 0, 1)).reshape(128, -1).view(bf16)
        rtr = np.ascontiguousarray(rc.transpose(2, 0, 1)).reshape(128, -1).view(bf16)

        xm = np.empty((128, gpc * C, D + 1), dtype=np.uint16)
        xm[:, :, :D] = xc.transpose(1, 0, 2)
        xm[:, :, D] = _BF16_ONE
        xm = xm.reshape(128, -1).view(bf16)

        ixc = np.ascontiguousarray(idx_rel[sl].reshape(gpc * C, 128).T)

        in_maps.append(
            {
                "xtr": xtr,
                "rtr": rtr,
                "xrm": xm,
                "idxg": ixc,
                "wco": wco,
                "io2": io2,
            }
        )
    return in_maps, gpc, C, b_val


def _run(in_maps, gpc, C, trace=False):
    from concourse.bass_utils import run_bass_kernel_spmd

    nc = _get_graph(gpc, C)
    res = run_bass_kernel_spmd(
        nc, in_maps, core_ids=list(range(N_CORES)), trace=trace
    )
    raw = np.concatenate(
        [res.results[i]["out"] for i in range(N_CORES)], axis=0
    ).astype(np.float64)
    s = raw[:, 0:129] + raw[:, 129:258]  # merge the two psum banks
    full = (s[:, :128] / (s[:, 128:129] + 1e-16)).astype(np.float32)
    return full, res


def kernel(x, ref, index, batch_size, W, b):
    in_maps, gpc, C, b_val = _prepare_inputs(x, ref, index, batch_size, W, b)
    assert b_val == 0.0, "nonzero bias not supported by this build"
    full, _ = _run(in_maps, gpc, C, trace=False)
    return full


# revision 37
# speedup vs baseline: 1.0866x; 1.0866x over previous
"""Trainium2 Bass kernel for segment-softmax attention (segment_reduce).

Computes, for row-sorted segment ids `index` (N rows, B segments):
    src  = tanh([x, ref] @ W + b)            # [N, 1]
    w    = segment_softmax(src, index)       # [N, 1]
    out  = segment_sum(w * x, index)         # [B, D]

Strategy (8 NeuronCores, SPMD, no collectives):
  - B segments are split into groups of 128; each core owns B/128/8
    contiguous groups, so shard boundaries align to segment boundaries
    and no cross-core reduction is needed.  Group row-ranges come from
    the host (sorted index), padded to a common chunk count C.
  - src = tanh(.) is in (-1,1), so exp never overflows and the segment
    max subtraction is dropped (identical up to float rounding).
  - Per 128-row chunk k of a group (on device):
      PE:  src column = Xt_k.T @ W1 + Rt_k.T @ W2        (psum [128,1])
      ACT: e = exp(tanh(src)) batched per group
      DVE: A[n,s] = e[n] * (idx[n] == s)  (one fused tensor_scalar)
      PE:  psum[128 segs, 129] += A.T @ [X_k | 1]        (col 128 = Z)
    evacuation: out = psum[:, :128] / (Z + 1e-16)  (DVE recip + ACT scale)
  - Value matmuls of group i run interleaved with the matvec matmuls of
    group i+2 (2-ahead software pipeline); psum accumulation alternates
    between two banks to keep consecutive matmuls pipelined.
  - Inputs are pre-quantized to bf16 on the host in the two layouts the
    PE needs (chunk-transposed for the matvec, row-major+ones column for
    the value pass); halves DMA traffic, rel-err ~3e-3 vs f32 reference.
"""

import numpy as np

N_CORES = 8
D = 128
SEG_PER_GROUP = 128  # psum partition dim = segments per group

_BF16_ONE = np.uint16(0x3F80)


def _f32_to_bf16_u16(a: np.ndarray) -> np.ndarray:
    """Round-to-nearest f32 -> bf16 bit pattern (uint16)."""
    a = np.ascontiguousarray(a, dtype=np.float32)
    u = a.view(np.uint32)
    rnd = ((u >> 16) & 1) + np.uint32(0x7FFF)
    return ((u + rnd) >> 16).astype(np.uint16)


def _build_graph(gpc: int, c_chunks: int):
    """Build the SPMD single-core graph (identical on all 8 cores)."""
    import concourse.bacc as bacc
    import concourse.mybir as mybir
    from concourse import tile
    from concourse.tile import add_dep_helper
    from contextlib import ExitStack

    dt = mybir.dt
    AF = mybir.ActivationFunctionType
    ALU = mybir.AluOpType

    C = c_chunks
    GC = gpc * C  # total chunks per core

    nc = bacc.Bacc(
        "TRN2",
        target_bir_lowering=False,
        debug=False,
        num_devices=N_CORES,
    )

    xtr = nc.dram_tensor("xtr", [128, GC * 128], dt.bfloat16, kind="ExternalInput").ap()
    rtr = nc.dram_tensor("rtr", [128, GC * 128], dt.bfloat16, kind="ExternalInput").ap()
    xrm = nc.dram_tensor("xrm", [128, GC * 129], dt.bfloat16, kind="ExternalInput").ap()
    idxg = nc.dram_tensor("idxg", [128, GC], dt.float32, kind="ExternalInput").ap()
    wco = nc.dram_tensor("wco", [128, 2], dt.bfloat16, kind="ExternalInput").ap()
    io2 = nc.dram_tensor("io2", [128, 128], dt.bfloat16, kind="ExternalInput").ap()
    out = nc.dram_tensor(
        "out", [gpc * SEG_PER_GROUP, D], dt.float32, kind="ExternalOutput"
    ).ap()

    with tile.TileContext(nc) as tc, ExitStack() as ctx:
        cpool = ctx.enter_context(tc.tile_pool(name="consts", bufs=1))
        xtp = ctx.enter_context(tc.tile_pool(name="xtp", bufs=2))
        rtp = ctx.enter_context(tc.tile_pool(name="rtp", bufs=2))
        xmp = ctx.enter_context(tc.tile_pool(name="xmp", bufs=3))
        epool = ctx.enter_context(tc.tile_pool(name="e", bufs=3))
        apool = ctx.enter_context(tc.tile_pool(name="amat", bufs=24))
        opool = ctx.enter_context(tc.tile_pool(name="osb", bufs=4))
        zpool = ctx.enter_context(tc.tile_pool(name="zr", bufs=4))
        ps_s = ctx.enter_context(tc.tile_pool(name="pss", bufs=2, space="PSUM"))
        ps_o = ctx.enter_context(tc.tile_pool(name="pso", bufs=6, space="PSUM"))

        wt = cpool.tile([128, 2], dt.bfloat16)
        nc.sync.dma_start(wt[:], wco[:])
        it = cpool.tile([128, 128], dt.bfloat16)
        nc.sync.dma_start(it[:], io2[:])
        # whole per-core index array resident in SBUF (2KB/partition)
        ixall = cpool.tile([128, GC], dt.float32)
        nc.sync.dma_start(ixall[:], idxg[:])
        ixneg = cpool.tile([128, GC], dt.float32)
        nc.vector.tensor_scalar(ixneg[:], ixall[:], -1.0, None, op0=ALU.mult)

        st = {}  # live tiles per pipeline stage

        def emit_load_and_src(g):
            xt = xtp.tile([128, C * 128], dt.bfloat16, tag="xt")
            nc.sync.dma_start(xt[:], xtr[:, g * C * 128:(g + 1) * C * 128])
            rt = rtp.tile([128, C * 128], dt.bfloat16, tag="rt")
            nc.sync.dma_start(rt[:], rtr[:, g * C * 128:(g + 1) * C * 128])
            xm = xmp.tile([128, C * 129], dt.bfloat16, tag="xm")
            nc.sync.dma_start(xm[:], xrm[:, g * C * 129:(g + 1) * C * 129])
            src = ps_s.tile([128, C], dt.float32, tag="src")
            st[g] = dict(xt=xt, rt=rt, xm=xm, src=src)

        def emit_src_chunk(g, k, after=None):
            s = st[g]
            mm = nc.tensor.matmul(
                s["src"][:, k:k + 1],
                s["xt"][:, k * 128:(k + 1) * 128],
                wt[:, 0:1],
                start=(k == 0),
                stop=False,
            )
            if after is not None:
                # ordering-only edge: spread the matvec matmuls between the
                # value matmuls instead of clustering at group boundaries
                add_dep_helper(mm.ins, after.ins, sync=False, reason="interleave")
            nc.tensor.matmul(
                s["src"][:, k:k + 1],
                s["rt"][:, k * 128:(k + 1) * 128],
                wt[:, 1:2],
                start=False,
                stop=(k == C - 1),
            )

        def emit_act(g):
            s = st[g]
            th = epool.tile([128, C], dt.float32, tag="th")
            nc.scalar.activation(th[:], s["src"][:], AF.Tanh)
            ee = epool.tile([128, C], dt.float32, tag="ee")
            nc.scalar.activation(ee[:], th[:], AF.Exp)
            s["ee"] = ee
            s["th"] = th

        def emit_po_alloc(g):
            # two psum banks alternate per chunk so consecutive accumulating
            # matmuls never target the same bank (keeps fill/drain pipelined)
            st[g]["po"] = [
                ps_o.tile([128, 129], dt.float32, tag="po", name="po"),
                ps_o.tile([128, 129], dt.float32, tag="po", name="po"),
            ]

        def emit_val_chunk(g, k):
            s = st[g]
            amat = apool.tile([128, 128], dt.bfloat16, tag="amat")
            if k % 4 == 3:
                # offload to the scalar engine: A = exp(th - 30*(iota-idx)^2)
                # = e * onehot(idx) up to ~1e-13 contamination
                u = apool.tile([128, 128], dt.bfloat16, tag="usq", name="usq")
                nc.scalar.activation(
                    u[:], it[:], AF.Square,
                    bias=ixneg[:, g * C + k:g * C + k + 1],
                )
                nc.scalar.activation(
                    amat[:], u[:], AF.Exp,
                    bias=s["th"][:, k:k + 1], scale=-30.0,
                )
            else:
                nc.vector.tensor_scalar(
                    amat[:],
                    it[:],
                    ixall[:, g * C + k:g * C + k + 1],
                    s["ee"][:, k:k + 1],
                    op0=ALU.is_equal,
                    op1=ALU.mult,
                )
            return nc.tensor.matmul(
                s["po"][k % 2][:],
                amat[:],
                s["xm"][:, k * 129:(k + 1) * 129],
                start=(k < 2),
                stop=(k >= C - 2),
            )

        def emit_evac(g):
            # bank-merge copy + final scale on the (mostly idle) scalar
            # engine; only add + reciprocal on the DVE critical chain
            s = st.pop(g)
            po_a, po_b = s["po"]
            ps = epool.tile([128, 129], dt.float32, tag="ps", name="ps")
            nc.scalar.copy(ps[:], po_a[:])
            nc.vector.tensor_add(ps[:], ps[:], po_b[:])
            ze = zpool.tile([128, 1], dt.float32, tag="ze")
            nc.vector.tensor_scalar(ze[:], ps[:, 128:129], 1e-16, None, op0=ALU.add)
            zi = zpool.tile([128, 1], dt.float32, tag="zi")
            nc.vector.reciprocal(zi[:], ze[:])
            ob = opool.tile([128, 128], dt.float32, tag="ob")
            nc.scalar.activation(ob[:], ps[:, 0:128], AF.Copy, scale=zi[:])
            nc.sync.dma_start(
                out[g * SEG_PER_GROUP:(g + 1) * SEG_PER_GROUP, :], ob[:]
            )

        # 2-ahead software pipeline: group i's value pass overlaps group
        # (i+2)'s load+matvec, so e(i+1) is always ready when the value
        # pass advances.
        for g in (0, 1):
            if g < gpc:
                emit_load_and_src(g)
                for k in range(C):
                    emit_src_chunk(g, k)
                emit_act(g)
        for i in range(gpc):
            emit_po_alloc(i)
            if i + 2 < gpc:
                emit_load_and_src(i + 2)
            last_vmm = None
            for k in range(C):
                if i + 2 < gpc:
                    emit_src_chunk(i + 2, k, after=last_vmm)
                last_vmm = emit_val_chunk(i, k)
            if i + 2 < gpc:
                emit_act(i + 2)
            emit_evac(i)

    nc.compile()
    return nc


_GRAPH_CACHE: dict = {}


def _get_graph(gpc: int, c_chunks: int):
    key = (gpc, c_chunks)
    if key not in _GRAPH_CACHE:
        _GRAPH_CACHE[key] = _build_graph(gpc, c_chunks)
    return _GRAPH_CACHE[key]


def _prepare_inputs(x, ref, index, batch_size, W, b):
    """Host-side sharding: group-aligned padding + bf16 layouts per core."""
    import concourse.mybir as mybir

    bf16 = mybir.dt.np(mybir.dt.bfloat16)

    x = np.ascontiguousarray(np.asarray(x, dtype=np.float32))
    ref = np.ascontiguousarray(np.asarray(ref, dtype=np.float32))
    idx = np.asarray(index).astype(np.int64).ravel()
    W = np.asarray(W, dtype=np.float32).reshape(-1)
    b_val = float(np.asarray(b, dtype=np.float32).reshape(-1)[0])

    n, d = x.shape
    assert d == D
    B = int(batch_size)
    ngroups = B // SEG_PER_GROUP
    assert B % SEG_PER_GROUP == 0 and ngroups % N_CORES == 0
    gpc = ngroups // N_CORES

    bounds = np.searchsorted(idx, np.arange(0, B + 1, SEG_PER_GROUP))
    rows_g = np.diff(bounds)
    C = max(1, int(np.ceil(rows_g.max() / 128)))
    R = C * 128

    offs = np.arange(R)[None, :]
    gidx = bounds[:-1, None] + offs  # [NG, R]
    valid = offs < rows_g[:, None]
    gidx_c = np.where(valid, np.minimum(gidx, n - 1), 0)

    # group-relative segment id; padding rows get 300 (never matches 0..127)
    idx_rel = np.where(
        valid,
        idx[gidx_c] - (np.arange(ngroups) * SEG_PER_GROUP)[:, None],
        300,
    ).astype(np.float32)

    xg = _f32_to_bf16_u16(x[gidx_c])  # [NG, R, D] u16
    rg = _f32_to_bf16_u16(ref[gidx_c])

    wco = np.zeros((128, 2), dtype=np.uint16)
    wco[:, 0] = _f32_to_bf16_u16(W[:128])
    wco[:, 1] = _f32_to_bf16_u16(W[128:256])
    wco = wco.view(bf16)

    io2 = np.broadcast_to(
        _f32_to_bf16_u16(np.arange(128, dtype=np.float32))[None, :], (128, 128)
    )
    io2 = np.ascontiguousarray(io2).view(bf16)

    in_maps = []
    for cid in range(N_CORES):
        sl = slice(cid * gpc, (cid + 1) * gpc)
        xc = xg[sl].reshape(gpc * C, 128, D)  # [chunks, row, d] u16
        rc = rg[sl].reshape(gpc * C, 128, D)

        xtr = np.ascontiguousarray(xc.transpose(2, 0, 1)).reshape(128, -1).view(bf16)
        rtr = np.ascontiguousarray(rc.transpose(2, 0, 1)).reshape(128, -1).view(bf16)

        xm = np.empty((128, gpc * C, D + 1), dtype=np.uint16)
        xm[:, :, :D] = xc.transpose(1, 0, 2)
        xm[:, :, D] = _BF16_ONE
        xm = xm.reshape(128, -1).view(bf16)

        ixc = np.ascontiguousarray(idx_rel[sl].reshape(gpc * C, 128).T)

        in_maps.append(
            {
                "xtr": xtr,
                "rtr": rtr,
                "xrm": xm,
                "idxg": ixc,
                "wco": wco,
                "io2": io2,
            }
        )
    return in_maps, gpc, C, b_val


def _run(in_maps, gpc, C, trace=False):
    from concourse.bass_utils import run_bass_kernel_spmd

    nc = _get_graph(gpc, C)
    res = run_bass_kernel_spmd(
        nc, in_maps, core_ids=list(range(N_CORES)), trace=trace
    )
    outs = [res.results[i]["out"] for i in range(N_CORES)]
    full = np.concatenate(outs, axis=0).astype(np.float32)
    return full, res


def kernel(x, ref, index, batch_size, W, b):
    in_maps, gpc, C, b_val = _prepare_inputs(x, ref, index, batch_size, W, b)
    assert b_val == 0.0, "nonzero bias not supported by this build"
    full, _ = _run(in_maps, gpc, C, trace=False)
    return full


# revision 38
# speedup vs baseline: 1.1824x; 1.0882x over previous
"""Trainium2 Bass kernel for segment-softmax attention (segment_reduce).

Computes, for row-sorted segment ids `index` (N rows, B segments):
    src  = tanh([x, ref] @ W + b)            # [N, 1]
    w    = segment_softmax(src, index)       # [N, 1]
    out  = segment_sum(w * x, index)         # [B, D]

Strategy (8 NeuronCores, SPMD, no collectives):
  - B segments are split into groups of 128; each core owns B/128/8
    contiguous groups, so shard boundaries align to segment boundaries
    and no cross-core reduction is needed.  Group row-ranges come from
    the host (sorted index), padded to a common chunk count C.
  - src = tanh(.) is in (-1,1), so exp never overflows and the segment
    max subtraction is dropped (identical up to float rounding).
  - Per 128-row chunk k of a group (on device):
      PE:  src column = Xt_k.T @ W1 + Rt_k.T @ W2        (psum [128,1])
      ACT: e = exp(tanh(src)) batched per group
      DVE: A[n,s] = e[n] * (idx[n] == s)  (one fused tensor_scalar)
      PE:  psum[128 segs, 129] += A.T @ [X_k | 1]        (col 128 = Z)
    evacuation: out = psum[:, :128] / (Z + 1e-16)  (DVE recip + ACT scale)
  - Value matmuls of group i run interleaved with the matvec matmuls of
    group i+2 (2-ahead software pipeline); psum accumulation alternates
    between two banks to keep consecutive matmuls pipelined.
  - Inputs are pre-quantized to bf16 on the host in the two layouts the
    PE needs (chunk-transposed for the matvec, row-major+ones column for
    the value pass); halves DMA traffic, rel-err ~3e-3 vs f32 reference.
"""

import numpy as np

N_CORES = 8
D = 128
SEG_PER_GROUP = 128  # psum partition dim = segments per group

_BF16_ONE = np.uint16(0x3F80)


def _f32_to_bf16_u16(a: np.ndarray) -> np.ndarray:
    """Round-to-nearest f32 -> bf16 bit pattern (uint16)."""
    a = np.ascontiguousarray(a, dtype=np.float32)
    u = a.view(np.uint32)
    rnd = ((u >> 16) & 1) + np.uint32(0x7FFF)
    return ((u + rnd) >> 16).astype(np.uint16)


def _build_graph(gpc: int, c_chunks: int):
    """Build the SPMD single-core graph (identical on all 8 cores)."""
    import concourse.bacc as bacc
    import concourse.mybir as mybir
    from concourse import tile
    from concourse.tile import add_dep_helper
    from contextlib import ExitStack

    dt = mybir.dt
    AF = mybir.ActivationFunctionType
    ALU = mybir.AluOpType

    C = c_chunks
    GC = gpc * C  # total chunks per core

    nc = bacc.Bacc(
        "TRN2",
        target_bir_lowering=False,
        debug=False,
        num_devices=N_CORES,
    )

    xtr = nc.dram_tensor("xtr", [128, GC * 128], dt.bfloat16, kind="ExternalInput").ap()
    rtr = nc.dram_tensor("rtr", [128, GC * 128], dt.bfloat16, kind="ExternalInput").ap()
    xrm = nc.dram_tensor("xrm", [128, GC * 129], dt.bfloat16, kind="ExternalInput").ap()
    idxg = nc.dram_tensor("idxg", [128, GC], dt.float32, kind="ExternalInput").ap()
    wco = nc.dram_tensor("wco", [128, 2], dt.bfloat16, kind="ExternalInput").ap()
    io2 = nc.dram_tensor("io2", [128, 128], dt.bfloat16, kind="ExternalInput").ap()
    out = nc.dram_tensor(
        "out", [gpc * SEG_PER_GROUP, D], dt.float32, kind="ExternalOutput"
    ).ap()

    with tile.TileContext(nc) as tc, ExitStack() as ctx:
        cpool = ctx.enter_context(tc.tile_pool(name="consts", bufs=1))
        xtp = ctx.enter_context(tc.tile_pool(name="xtp", bufs=2))
        rtp = ctx.enter_context(tc.tile_pool(name="rtp", bufs=2))
        xmp = ctx.enter_context(tc.tile_pool(name="xmp", bufs=3))
        epool = ctx.enter_context(tc.tile_pool(name="e", bufs=3))
        apool = ctx.enter_context(tc.tile_pool(name="amat", bufs=24))
        opool = ctx.enter_context(tc.tile_pool(name="osb", bufs=4))
        zpool = ctx.enter_context(tc.tile_pool(name="zr", bufs=4))
        ps_s = ctx.enter_context(tc.tile_pool(name="pss", bufs=2, space="PSUM"))
        ps_o = ctx.enter_context(tc.tile_pool(name="pso", bufs=6, space="PSUM"))

        wt = cpool.tile([128, 2], dt.bfloat16)
        nc.sync.dma_start(wt[:], wco[:])
        it = cpool.tile([128, 128], dt.bfloat16)
        nc.sync.dma_start(it[:], io2[:])
        # whole per-core index array resident in SBUF (2KB/partition)
        ixall = cpool.tile([128, GC], dt.float32)
        nc.sync.dma_start(ixall[:], idxg[:])
        ixneg = cpool.tile([128, GC], dt.float32)
        nc.vector.tensor_scalar(ixneg[:], ixall[:], -1.0, None, op0=ALU.mult)

        st = {}  # live tiles per pipeline stage

        def emit_load_and_src(g):
            xt = xtp.tile([128, C * 128], dt.bfloat16, tag="xt")
            nc.sync.dma_start(xt[:], xtr[:, g * C * 128:(g + 1) * C * 128])
            rt = rtp.tile([128, C * 128], dt.bfloat16, tag="rt")
            nc.sync.dma_start(rt[:], rtr[:, g * C * 128:(g + 1) * C * 128])
            xm = xmp.tile([128, C * 129], dt.bfloat16, tag="xm")
            nc.sync.dma_start(xm[:], xrm[:, g * C * 129:(g + 1) * C * 129])
            src = ps_s.tile([128, C], dt.float32, tag="src")
            st[g] = dict(xt=xt, rt=rt, xm=xm, src=src)

        def emit_src_chunk(g, k, after=None):
            s = st[g]
            mm = nc.tensor.matmul(
                s["src"][:, k:k + 1],
                s["xt"][:, k * 128:(k + 1) * 128],
                wt[:, 0:1],
                start=(k == 0),
                stop=False,
            )
            if after is not None:
                # ordering-only edge: spread the matvec matmuls between the
                # value matmuls instead of clustering at group boundaries
                add_dep_helper(mm.ins, after.ins, sync=False, reason="interleave")
            nc.tensor.matmul(
                s["src"][:, k:k + 1],
                s["rt"][:, k * 128:(k + 1) * 128],
                wt[:, 1:2],
                start=False,
                stop=(k == C - 1),
            )

        def emit_act(g):
            s = st[g]
            th = epool.tile([128, C], dt.float32, tag="th")
            nc.scalar.activation(th[:], s["src"][:], AF.Tanh)
            ee = epool.tile([128, C], dt.float32, tag="ee")
            nc.scalar.activation(ee[:], th[:], AF.Exp)
            s["ee"] = ee
            s["th"] = th

        def emit_po_alloc(g):
            # two psum banks alternate per chunk so consecutive accumulating
            # matmuls never target the same bank (keeps fill/drain pipelined)
            st[g]["po"] = [
                ps_o.tile([128, 129], dt.float32, tag="po", name="po"),
                ps_o.tile([128, 129], dt.float32, tag="po", name="po"),
            ]

        def emit_val_chunk(g, k):
            s = st[g]
            amat = apool.tile([128, 128], dt.bfloat16, tag="amat")
            if k % 3 == 2:
                # offload to the scalar engine: A = exp(th - 30*(iota-idx)^2)
                # = e * onehot(idx) up to ~1e-13 contamination
                u = apool.tile([128, 128], dt.bfloat16, tag="usq", name="usq")
                nc.scalar.activation(
                    u[:], it[:], AF.Square,
                    bias=ixneg[:, g * C + k:g * C + k + 1],
                )
                nc.scalar.activation(
                    amat[:], u[:], AF.Exp,
                    bias=s["th"][:, k:k + 1], scale=-30.0,
                )
            else:
                nc.vector.tensor_scalar(
                    amat[:],
                    it[:],
                    ixall[:, g * C + k:g * C + k + 1],
                    s["ee"][:, k:k + 1],
                    op0=ALU.is_equal,
                    op1=ALU.mult,
                )
            return nc.tensor.matmul(
                s["po"][k % 2][:],
                amat[:],
                s["xm"][:, k * 129:(k + 1) * 129],
                start=(k < 2),
                stop=(k >= C - 2),
            )

        def emit_evac(g):
            # bank-merge copy + final scale on the (mostly idle) scalar
            # engine; only add + reciprocal on the DVE critical chain
            s = st.pop(g)
            po_a, po_b = s["po"]
            ps = epool.tile([128, 129], dt.float32, tag="ps", name="ps")
            nc.scalar.copy(ps[:], po_a[:])
            nc.vector.tensor_add(ps[:], ps[:], po_b[:])
            ze = zpool.tile([128, 1], dt.float32, tag="ze")
            nc.vector.tensor_scalar(ze[:], ps[:, 128:129], 1e-16, None, op0=ALU.add)
            zi = zpool.tile([128, 1], dt.float32, tag="zi")
            nc.vector.reciprocal(zi[:], ze[:])
            ob = opool.tile([128, 128], dt.float32, tag="ob")
            nc.scalar.activation(ob[:], ps[:, 0:128], AF.Copy, scale=zi[:])
            nc.sync.dma_start(
                out[g * SEG_PER_GROUP:(g + 1) * SEG_PER_GROUP, :], ob[:]
            )

        # 2-ahead software pipeline: group i's value pass overlaps group
        # (i+2)'s load+matvec, so e(i+1) is always ready when the value
        # pass advances.
        for g in (0, 1):
            if g < gpc:
                emit_load_and_src(g)
                for k in range(C):
                    emit_src_chunk(g, k)
                emit_act(g)
        for i in range(gpc):
            emit_po_alloc(i)
            if i + 2 < gpc:
                emit_load_and_src(i + 2)
            last_vmm = None
            for k in range(C):
                if i + 2 < gpc:
                    emit_src_chunk(i + 2, k, after=last_vmm)
                last_vmm = emit_val_chunk(i, k)
            if i + 2 < gpc:
                emit_act(i + 2)
            emit_evac(i)

    nc.compile()
    return nc


_GRAPH_CACHE: dict = {}


def _get_graph(gpc: int, c_chunks: int):
    key = (gpc, c_chunks)
    if key not in _GRAPH_CACHE:
        _GRAPH_CACHE[key] = _build_graph(gpc, c_chunks)
    return _GRAPH_CACHE[key]


def _prepare_inputs(x, ref, index, batch_size, W, b):
    """Host-side sharding: group-aligned padding + bf16 layouts per core."""
    import concourse.mybir as mybir

    bf16 = mybir.dt.np(mybir.dt.bfloat16)

    x = np.ascontiguousarray(np.asarray(x, dtype=np.float32))
    ref = np.ascontiguousarray(np.asarray(ref, dtype=np.float32))
    idx = np.asarray(index).astype(np.int64).ravel()
    W = np.asarray(W, dtype=np.float32).reshape(-1)
    b_val = float(np.asarray(b, dtype=np.float32).reshape(-1)[0])

    n, d = x.shape
    assert d == D
    B = int(batch_size)
    ngroups = B // SEG_PER_GROUP
    assert B % SEG_PER_GROUP == 0 and ngroups % N_CORES == 0
    gpc = ngroups // N_CORES

    bounds = np.searchsorted(idx, np.arange(0, B + 1, SEG_PER_GROUP))
    rows_g = np.diff(bounds)
    C = max(1, int(np.ceil(rows_g.max() / 128)))
    R = C * 128

    offs = np.arange(R)[None, :]
    gidx = bounds[:-1, None] + offs  # [NG, R]
    valid = offs < rows_g[:, None]
    gidx_c = np.where(valid, np.minimum(gidx, n - 1), 0)

    # group-relative segment id; padding rows get 300 (never matches 0..127)
    idx_rel = np.where(
        valid,
        idx[gidx_c] - (np.arange(ngroups) * SEG_PER_GROUP)[:, None],
        300,
    ).astype(np.float32)

    xg = _f32_to_bf16_u16(x[gidx_c])  # [NG, R, D] u16
    rg = _f32_to_bf16_u16(ref[gidx_c])

    wco = np.zeros((128, 2), dtype=np.uint16)
    wco[:, 0] = _f32_to_bf16_u16(W[:128])
    wco[:, 1] = _f32_to_bf16_u16(W[128:256])
    wco = wco.view(bf16)

    io2 = np.broadcast_to(
        _f32_to_bf16_u16(np.arange(128, dtype=np.float32))[None, :], (128, 128)
    )
    io2 = np.ascontiguousarray(io2).view(bf16)

    in_maps = []
    for cid in range(N_CORES):
        sl = slice(cid * gpc, (cid + 1) * gpc)
        xc = xg[sl].reshape(gpc * C, 128, D)  # [chunks, row, d] u16
        rc = rg[sl].reshape(gpc * C, 128, D)

        xtr = np.ascontiguousarray(xc.transpose(2, 0, 1)).reshape(128, -1).view(bf16)
        rtr = np.ascontiguousarray(rc.transpose(2, 0, 1)).reshape(128, -1).view(bf16)

        xm = np.empty((128, gpc * C, D + 1), dtype=np.uint16)
        xm[:, :, :D] = xc.transpose(1, 0, 2)
        xm[:, :, D] = _BF16_ONE
        xm = xm.reshape(128, -1).view(bf16)

        ixc = np.ascontiguousarray(idx_rel[sl].reshape(gpc * C, 128).T)

        in_maps.append(
            {
                "xtr": xtr,
                "rtr": rtr,
                "xrm": xm,
                "idxg": ixc,
                "wco": wco,
                "io2": io2,
            }
        )
    return in_maps, gpc, C, b_val


def _run(in_maps, gpc, C, trace=False):
    from concourse.bass_utils import run_bass_kernel_spmd

    nc = _get_graph(gpc, C)
    res = run_bass_kernel_spmd(
        nc, in_maps, core_ids=list(range(N_CORES)), trace=trace
    )
    outs = [res.results[i]["out"] for i in range(N_CORES)]
    full = np.concatenate(outs, axis=0).astype(np.float32)
    return full, res


def kernel(x, ref, index, batch_size, W, b):
    in_maps, gpc, C, b_val = _prepare_inputs(x, ref, index, batch_size, W, b)
    assert b_val == 0.0, "nonzero bias not supported by this build"
    full, _ = _run(in_maps, gpc, C, trace=False)
    return full
